# revision 4
# baseline (speedup 1.0000x reference)
"""Trainium2 Bass kernel for nn_CrossAttention_51539607552970.

Sharding: 8 cores = 2 (batch) x 4 (GQA kv-head groups). Each core computes
4 query heads + its single kv head for one batch element, producing a
partial output (its head-group's contribution through wo); the host sums
the 4 partials per batch element (tensor-parallel unshard).

On-device layout is feature-major: the host passes x/c transposed
([hid, tokens]) so every matmul contracts the partition dimension
natively.  Scores are computed transposed ([keys, q]) so the softmax
denominator is a PE ones-matmul that accumulates into the same PSUM bank
as the P@V accumulator (columns 256:512 vs 0:256).  All matmuls run in
float32r (full-rate fp32 mode, ~1e-4 precision).
"""

import sys

sys.path.insert(0, "/opt/trn_rl_repo")

import numpy as np

import concourse.bass as bass
import concourse.mybir as mybir
import concourse.tile as tile
from concourse import bacc
from concourse.bass_utils import run_bass_kernel_spmd
from concourse.masks import make_identity

F32 = mybir.dt.float32
F32R = mybir.dt.float32r
AF = mybir.ActivationFunctionType
OP = mybir.AluOpType

# Problem constants (hardcoded per contract).
B, S, L = 2, 2048, 2048
H, KVH, D = 16, 4, 128
HID = H * D
EPS = 1e-6
SCALE = 1.0 / np.sqrt(D)

NH = 4           # query heads per core
P = 128          # partitions
HC = HID // P    # 16 hid chunks
KC = L // P      # 16 key chunks
PB = 512         # projection block width (tokens)
AB = 256         # attention block width (queries)
NPB = S // PB    # 4
NAB = S // AB    # 8

_compiled = None


def _build():
    nc = bacc.Bacc("TRN2", num_devices=8)

    xT = nc.dram_tensor("xT", [HID, S], F32R, kind="ExternalInput")
    cT = nc.dram_tensor("cT", [HID, L], F32R, kind="ExternalInput")
    wq = nc.dram_tensor("wq", [HID, NH * D], F32R, kind="ExternalInput")
    wk = nc.dram_tensor("wk", [HID, D], F32R, kind="ExternalInput")
    wv = nc.dram_tensor("wv", [HID, D], F32R, kind="ExternalInput")
    wo = nc.dram_tensor("wo", [NH * D, HID], F32R, kind="ExternalInput")
    nqw = nc.dram_tensor("nqw", [P, 1], F32, kind="ExternalInput")
    nkw = nc.dram_tensor("nkw", [P, 1], F32, kind="ExternalInput")
    out = nc.dram_tensor("out", [S, HID], F32, kind="ExternalOutput")

    with nc.allow_low_precision(reason="f32r matmul input rounding"), \
         tile.TileContext(nc) as tc:
        with tc.tile_pool(name="consts", bufs=1) as consts, \
             tc.tile_pool(name="weights", bufs=1) as weights, \
             tc.tile_pool(name="stream", bufs=12) as stream, \
             tc.tile_pool(name="kv", bufs=1) as kvpool, \
             tc.tile_pool(name="xqt", bufs=1) as xqtpool, \
             tc.tile_pool(name="small", bufs=2) as small, \
             tc.tile_pool(name="esbp", bufs=6) as esbp, \
             tc.tile_pool(name="outp", bufs=4) as outp, \
             tc.tile_pool(name="psum", bufs=1, space="PSUM") as psum:

            # ---- constants ----
            ones_f = consts.tile([P, P], F32)
            nc.vector.memset(ones_f[:], 1.0)
            ones = consts.tile([P, P], F32R)
            nc.scalar.copy(ones[:], ones_f[:])
            ident = consts.tile([P, P], F32)
            make_identity(nc, ident)
            nqw_sb = consts.tile([P, 1], F32)
            nc.sync.dma_start(nqw_sb[:], nqw[:])
            nkw_sb = consts.tile([P, 1], F32)
            nc.sync.dma_start(nkw_sb[:], nkw[:])
            eps_sb = consts.tile([P, 1], F32)
            nc.vector.memset(eps_sb[:], EPS)

            # ---- resident weights ----
            wq_sb = weights.tile([P, HC * NH * D], F32R)   # 16 chunks x 512
            for hc in range(HC):
                nc.sync.dma_start(wq_sb[:, hc * 512:(hc + 1) * 512],
                                  wq[hc * P:(hc + 1) * P, :])
            wk_sb = weights.tile([P, HC * D], F32R)
            wv_sb = weights.tile([P, HC * D], F32R)
            for hc in range(HC):
                nc.sync.dma_start(wk_sb[:, hc * D:(hc + 1) * D],
                                  wk[hc * P:(hc + 1) * P, :])
                nc.sync.dma_start(wv_sb[:, hc * D:(hc + 1) * D],
                                  wv[hc * P:(hc + 1) * P, :])
            wo_sb = weights.tile([P, NH * HID], F32R)      # 4 head-chunks x 2048
            for h in range(NH):
                nc.sync.dma_start(wo_sb[:, h * HID:(h + 1) * HID],
                                  wo[h * P:(h + 1) * P, :])

            # ---- persistent activations ----
            kT_sb = kvpool.tile([P, L], F32R)              # [D, keys]
            v_sb = kvpool.tile([P, KC * D], F32R)          # kt-th block = [keys(kt), D]
            xqT_list = [xqtpool.tile([P, S], F32R, name=f"xqT{h}") for h in range(NH)]

            # =========== Phase B: K/V projections (stream cT) ===========
            for kcol in range(4):  # 512-wide key column blocks
                ct_tiles = []
                for hc in range(HC):
                    t = stream.tile([P, 512], F32R, name="ct", tag="stream")
                    nc.sync.dma_start(
                        t[:], cT[hc * P:(hc + 1) * P, kcol * 512:(kcol + 1) * 512])
                    ct_tiles.append(t)

                # kT/vT blocks: [D, 512 keys]; interleave per-chunk so each
                # streamed cT tile is consumed immediately by both matmuls.
                kps = psum.tile([P, 512], F32, name="kps", tag="combo", bufs=4)
                vps = psum.tile([P, 512], F32, name="vps", tag="combo", bufs=4)
                for hc in range(HC):
                    nc.tensor.matmul(kps[:], wk_sb[:, hc * D:(hc + 1) * D],
                                     ct_tiles[hc][:],
                                     start=(hc == 0), stop=(hc == HC - 1))
                    nc.tensor.matmul(vps[:], wv_sb[:, hc * D:(hc + 1) * D],
                                     ct_tiles[hc][:],
                                     start=(hc == 0), stop=(hc == HC - 1))
                vT_sb = small.tile([P, 512], F32, name="vT", tag="vT")
                nc.vector.tensor_copy(vT_sb[:], vps[:])
                # k rmsnorm over D (partition dim): sumsq via ones matmul
                ksq = small.tile([P, 512], F32R, name="ksq", tag="sq")
                nc.scalar.square(ksq[:], kps[:])
                ksum = psum.tile([P, 512], F32, name="ksum", tag="work", bufs=2)
                nc.tensor.matmul(ksum[:], ones[:], ksq[:], start=True, stop=True)
                krs = small.tile([P, 512], F32, name="krs", tag="rs")
                nc.scalar.activation(krs[:], ksum[:], AF.Sqrt,
                                     bias=eps_sb[:], scale=1.0 / D)
                krr = small.tile([P, 512], F32, name="krr", tag="rr")
                nc.vector.reciprocal(krr[:], krs[:])
                # kT = (kps * nkw) * rsqrt  (fused)
                nc.vector.scalar_tensor_tensor(
                    out=kT_sb[:, kcol * 512:(kcol + 1) * 512], in0=kps[:],
                    scalar=nkw_sb[:], in1=krr[:], op0=OP.mult, op1=OP.mult)
                # transpose 128x128 blocks -> v_sb [keys, D]
                for j in range(4):
                    kt = kcol * 4 + j
                    tp = psum.tile([P, P], F32, name="tp", tag="work", bufs=2)
                    nc.tensor.transpose(tp[:], vT_sb[:, j * P:(j + 1) * P], ident[:])
                    nc.vector.tensor_copy(v_sb[:, kt * D:(kt + 1) * D], tp[:])

            # =========== Phase A: Q projection (stream xT) ===========
            for pb in range(NPB):
                xt_tiles = []
                for hc in range(HC):
                    t = stream.tile([P, PB], F32R, name="xt", tag="stream")
                    nc.sync.dma_start(
                        t[:], xT[hc * P:(hc + 1) * P, pb * PB:(pb + 1) * PB])
                    xt_tiles.append(t)
                qpss = [psum.tile([P, PB], F32, name=f"qps{h}",
                                  tag="combo", bufs=4) for h in range(NH)]
                for hc in range(HC):
                    for h in range(NH):
                        nc.tensor.matmul(
                            qpss[h][:],
                            wq_sb[:, hc * 512 + h * D: hc * 512 + (h + 1) * D],
                            xt_tiles[hc][:],
                            start=(hc == 0), stop=(hc == HC - 1))
                for h in range(NH):
                    qps = qpss[h]
                    qsq = small.tile([P, PB], F32R, name="qsq", tag="sq")
                    nc.scalar.square(qsq[:], qps[:])
                    qsum = psum.tile([P, PB], F32, name="qsum", tag="work", bufs=2)
                    nc.tensor.matmul(qsum[:], ones[:], qsq[:], start=True, stop=True)
                    qrs = small.tile([P, PB], F32, name="qrs", tag="rs")
                    nc.scalar.activation(qrs[:], qsum[:], AF.Sqrt,
                                         bias=eps_sb[:], scale=1.0 / D)
                    qrr = small.tile([P, PB], F32, name="qrr", tag="rr")
                    nc.vector.reciprocal(qrr[:], qrs[:])
                    nc.vector.scalar_tensor_tensor(
                        out=xqT_list[h][:, pb * PB:(pb + 1) * PB], in0=qps[:],
                        scalar=nqw_sb[:], in1=qrr[:], op0=OP.mult, op1=OP.mult)

            # =========== Phase C: attention + wo ===========
            for ab in range(NAB):
                q0 = ab * AB
                combos = [psum.tile([P, 512], F32, name=f"combo{h}",
                                    tag="combo", bufs=4) for h in range(NH)]
                for kt in range(KC):
                    # scores (transposed): ST[h] = kT_kt^T @ xqT_h  -> [keys, q]
                    sts = []
                    for h in range(NH):
                        st = psum.tile([P, AB], F32, name="st", tag="st", bufs=2)
                        nc.tensor.matmul(st[:], kT_sb[:, kt * P:(kt + 1) * P],
                                         xqT_list[h][:, q0:q0 + AB],
                                         start=True, stop=True)
                        sts.append(st)
                    es = []
                    for h in range(NH):
                        e = esbp.tile([P, AB], F32R, name="e", tag="e")
                        nc.scalar.activation(e[:], sts[h][:], AF.Exp)
                        es.append(e)
                    # softmax denominators (replicated) into combo[:, 256:512].
                    # Only this group carries start=True: start clears
                    # has_written for the WHOLE bank, so the P@V group below
                    # must never re-clear it (kt=0 writes land as overwrites
                    # because the bits are already cleared).
                    for h in range(NH):
                        nc.tensor.matmul(combos[h][:, AB:2 * AB],
                                         ones[:], es[h][:],
                                         start=(kt == 0), stop=(kt == KC - 1),
                                         skip_group_check=True)
                    # P@V accumulate into combo[:, 0:256]
                    for h in range(NH):
                        nc.tensor.matmul(combos[h][:, 0:AB],
                                         v_sb[:, kt * D:(kt + 1) * D], es[h][:],
                                         start=False, stop=(kt == KC - 1),
                                         skip_group_check=True)
                # normalize: attn = combo[:, 0:AB] / sums
                attns = []
                for h in range(NH):
                    rr = small.tile([P, AB], F32, name="arr", tag="arr")
                    nc.vector.reciprocal(rr[:], combos[h][:, AB:2 * AB])
                    attn = small.tile([P, AB], F32R, name="attn",
                                      tag=f"attn{h}", bufs=2)
                    nc.vector.tensor_tensor(
                        out=attn[:], in0=combos[h][:, 0:AB], in1=rr[:],
                        op=OP.mult)
                    attns.append(attn)
                # wo: out[q, :] += attn_h^T @ wo_h for 128-row q-subtiles
                for qs in range(AB // P):  # 2
                    wops = [psum.tile([P, 512], F32, name=f"wop{ht}",
                                      tag="combo", bufs=4) for ht in range(4)]
                    for h in range(NH):
                        for ht in range(4):
                            nc.tensor.matmul(
                                wops[ht][:],
                                attns[h][:, qs * P:(qs + 1) * P],
                                wo_sb[:, h * HID + ht * 512: h * HID + (ht + 1) * 512],
                                start=(h == 0), stop=(h == NH - 1))
                    for ht in range(4):
                        ot = outp.tile([P, 512], F32, name="ot", tag="ot")
                        nc.vector.tensor_copy(ot[:], wops[ht][:])
                        nc.sync.dma_start(
                            out[q0 + qs * P: q0 + (qs + 1) * P,
                                ht * 512:(ht + 1) * 512], ot[:])

    nc.compile()
    return nc


def _get_compiled():
    global _compiled
    if _compiled is None:
        _compiled = _build()
    return _compiled


def _shard_inputs(x, c, wq, wkv, wo, norm_q_w, norm_k_w):
    x = np.asarray(x, np.float32)
    c = np.asarray(c, np.float32)
    wq = np.asarray(wq, np.float32)
    wkv = np.asarray(wkv, np.float32)
    wo = np.asarray(wo, np.float32)
    nqw = (np.asarray(norm_q_w, np.float32) * np.float32(SCALE)).reshape(P, 1)
    nkw = np.asarray(norm_k_w, np.float32).reshape(P, 1).copy()

    xTs = [np.ascontiguousarray(x[b].T) for b in range(B)]
    cTs = [np.ascontiguousarray(c[b].T) for b in range(B)]
    in_maps = []
    for core in range(8):
        b, g = core // 4, core % 4
        blk = wkv[:, g * 256:(g + 1) * 256]
        in_maps.append({
            "xT": xTs[b],
            "cT": cTs[b],
            "wq": np.ascontiguousarray(wq[:, g * 512:(g + 1) * 512]),
            "wk": np.ascontiguousarray(blk[:, 0::2]),
            "wv": np.ascontiguousarray(blk[:, 1::2]),
            "wo": np.ascontiguousarray(wo[g * 512:(g + 1) * 512, :]),
            "nqw": nqw,
            "nkw": nkw,
        })
    return in_maps


def run_sharded(inputs, trace=False, trace_cores=None):
    """Run the SPMD kernel; returns (full_output, BassKernelResults)."""
    nc = _get_compiled()
    in_maps = _shard_inputs(**inputs)
    res = run_bass_kernel_spmd(nc, in_maps, core_ids=list(range(8)),
                               trace=trace, trace_cores=trace_cores)
    parts = [r["out"] for r in res.results]
    full = np.empty((B, S, HID), np.float32)
    for b in range(B):
        full[b] = np.sum(np.stack([parts[4 * b + g] for g in range(4)], 0),
                         axis=0, dtype=np.float64).astype(np.float32)
    return full, res


def kernel(**inputs) -> np.ndarray:
    out, _ = run_sharded(inputs, trace=False)
    return out
